# revision 38
# baseline (speedup 1.0000x reference)
"""Trainium2 Bass kernel v2: pattern-compressed sparse-conv gather-GEMM-scatter.

Math: out[j, d] = sum over points i with out_idx[i]==j of  x[i, :] @ W[k_idx[i], :, d]

v1 ("dense k-slot expansion") sent 8 slots x 32ch = 256 values per output
voxel; at ~38% slot occupancy that wastes ~62% of HBM read traffic on zeros,
and the kernel sits at the per-core DMA roofline.

v2 ("pattern compression"): each output voxel j has an occupied-slot mask
(which of the 8 kernel offsets have a point). ~85% of voxels have <= 4
occupied slots. Those are packed as 4 slot-blocks (128 values) under one of
C(8,4)=70 canonical patterns; the matmul stationary for pattern p is the
[128, 32] stack of W[k] rows for p's slots (host-built dictionary). Voxels
with >= 5 slots use the full dense-256 layout (two accumulated matmuls).

Voxels are sorted by (class, pattern) and dealt round-robin across the 8
cores, so all cores share an identical compile-time program structure (runs
differ by <= 1 column, zero-padded). Output voxel order is the sorted order;
the host inverse-permutes after gather (free).

Per-core traffic: ~7.0 MB (A slab) + ~2.4 MB (B slab) + 0.57 MB (pattern
weight dict) in, ~2.5 MB out (bf16) -- vs 21 MB for v1. Tensor: ~37k PE
cycles vs 65.5k.
"""
import sys

if "/opt/trn_rl_repo" not in sys.path:
    sys.path.insert(0, "/opt/trn_rl_repo")

import numpy as np

N_CORES = 8
CHUNK = 4096        # slabA DMA chunk (columns)
PSUM_W = 512        # psum tile width (voxel columns)
STAGE_W = 2048      # staging tile [128, 2048] = 8192 voxel columns

POP = np.array([bin(i).count("1") for i in range(256)], dtype=np.int64)
PAT4 = [b for b in range(256) if POP[b] == 4]           # 70 patterns
PAT_ID = np.full(256, -1, dtype=np.int64)
for _i, _b in enumerate(PAT4):
    PAT_ID[_b] = _i

_prog_cache = {}


def _build_program(desc):
    import concourse.tile as tile
    from concourse import bacc, mybir

    bf16 = mybir.dt.bfloat16
    f32 = mybir.dt.float32
    NAc, LB, Ntot, nb = desc["NAc"], desc["LB"], desc["Ntot"], desc["nb"]
    segs = desc["segs"]
    nchunks = max(1, -(-NAc // CHUNK))

    nc = bacc.Bacc("TRN2", target_bir_lowering=False, debug=False)
    slabA_d = nc.dram_tensor("slabA", [128, max(NAc, 1)], bf16, kind="ExternalInput")
    slabB_d = nc.dram_tensor("slabB", [256, max(LB, 1)], bf16, kind="ExternalInput")
    wdict_d = nc.dram_tensor("wdict", [128, 70 * 32], bf16, kind="ExternalInput")
    wflat_d = nc.dram_tensor("wflat", [256, 32], bf16, kind="ExternalInput")
    out_d = nc.dram_tensor("out_st", [nb, 128, STAGE_W], bf16, kind="ExternalOutput")

    with tile.TileContext(nc) as tc:
        with (
            tc.tile_pool(name="w", bufs=1) as wpool,
            tc.tile_pool(name="a", bufs=1) as apool,
            tc.tile_pool(name="bb", bufs=1) as bpool,
            tc.tile_pool(name="st", bufs=2) as stpool,
            tc.tile_pool(name="ps", bufs=4, space="PSUM") as pspool,
        ):
            # one merged weight tile: 70 pattern stationaries + 2 wflat halves
            wdict_t = wpool.tile([128, 72 * 32], bf16, tag="wdict")
            wf0 = wdict_t[:, 70 * 32:71 * 32]
            wf1 = wdict_t[:, 71 * 32:72 * 32]
            nc.sync.dma_start(wdict_t[:, 0:70 * 32], wdict_d.ap()[:, :])
            nc.sync.dma_start(wf0, wflat_d.ap()[0:128, :])
            nc.sync.dma_start(wf1, wflat_d.ap()[128:256, :])

            # gpsimd queue: earliest-consumed chunks only, then output DMAs;
            # remaining input byte-balanced on sync/scalar
            qload = {"sync": 0.6e6, "scalar": 0.0}
            qeng = {"sync": nc.sync, "scalar": nc.scalar}
            chA = []
            for i in range(nchunks):
                lo = i * CHUNK
                hi = min(max(NAc, 1), lo + CHUNK)
                t = apool.tile([128, hi - lo], bf16, tag=f"ca{i}")
                if i < 2:
                    nc.gpsimd.dma_start(t[:], slabA_d.ap()[:, lo:hi])
                else:
                    qn = min(qload, key=qload.get)
                    qload[qn] += (hi - lo) * 256
                    qeng[qn].dma_start(t[:], slabA_d.ap()[:, lo:hi])
                chA.append(t)
            b0 = bpool.tile([128, max(LB, 1)], bf16, tag="b0")
            b1 = bpool.tile([128, max(LB, 1)], bf16, tag="b1")
            for half, bt in ((0, b0), (1, b1)):
                qn = min(qload, key=qload.get)
                qload[qn] += max(LB, 1) * 256
                qeng[qn].dma_start(bt[:], slabB_d.ap()[128 * half:128 * (half + 1), :])

            copy_i = 0
            for b in range(nb):
                staging = stpool.tile([128, STAGE_W], bf16)
                for g in range(4):
                    Tbase = b * 16 + 4 * g
                    if PSUM_W * Tbase >= Ntot:
                        continue  # tail: staging garbage, never gathered
                    ps = pspool.tile([128, PSUM_W], f32)
                    for a in range(4):
                        T = Tbase + a
                        lo_g = PSUM_W * T
                        if lo_g >= Ntot:
                            continue
                        for s in segs[T]:
                            if s[0] == 0:
                                _, o, w, pid, ci = s
                                off_in_chunk = lo_g + o - ci * CHUNK
                                nc.tensor.matmul(
                                    ps[32 * a:32 * a + 32, o:o + w],
                                    wdict_t[:, 32 * pid:32 * pid + 32],
                                    chA[ci][:, off_in_chunk:off_in_chunk + w],
                                    start=True, stop=True,
                                    tile_position=(0, 32 * a),
                                )
                            else:
                                _, o, w, boff = s
                                nc.tensor.matmul(ps[32 * a:32 * a + 32, o:o + w],
                                                 wf0, b0[:, boff:boff + w],
                                                 start=True, stop=False,
                                                 tile_position=(0, 32 * a))
                                nc.tensor.matmul(ps[32 * a:32 * a + 32, o:o + w],
                                                 wf1, b1[:, boff:boff + w],
                                                 start=False, stop=True,
                                                 tile_position=(0, 32 * a))
                    dst = staging[:, 512 * g:512 * g + 512]
                    if copy_i % 2 == 0:
                        nc.vector.tensor_copy(dst, ps[:])
                    else:
                        nc.scalar.copy(dst, ps[:])
                    copy_i += 1
                    if b == nb - 1:
                        # last block: per-group output pieces, alternating
                        # queues, so the final compute->copy->DMA chain is
                        # short and the last pieces drain in parallel
                        (nc.gpsimd if g % 2 else nc.sync).dma_start(
                            out_d.ap()[b][:, 512 * g:512 * g + 512], dst)
                    elif b == nb - 2 and g % 2 == 1:
                        nc.gpsimd.dma_start(
                            out_d.ap()[b][:, 512 * (g - 1):512 * (g + 1)],
                            staging[:, 512 * (g - 1):512 * (g + 1)])
                if b < nb - 2:
                    nc.gpsimd.dma_start(out_d.ap()[b], staging[:])

    nc.compile()
    return nc


def _get_program(desc):
    key = (desc["NAc"], desc["LB"], desc["Ntot"], desc["nb"],
           tuple(tuple(map(tuple, s)) for s in desc["segs"]))
    if key not in _prog_cache:
        _prog_cache[key] = _build_program(desc)
    return _prog_cache[key]


def _pack(x, W, k_idx, out_idx, num_out):
    """Host-side packing. Returns (in_maps, desc, vox_core, vox_col)."""
    import ml_dtypes
    bf = ml_dtypes.bfloat16
    n = x.shape[0]

    # occupied-slot masks per output voxel
    masks = np.zeros(num_out, np.uint8)
    np.bitwise_or.at(masks, out_idx, (np.uint8(1) << k_idx.astype(np.uint8)))
    m = POP[masks]

    # class A (m<=4): canonical 4-bit pattern = mask padded with lowest unset bits
    isA = m <= 4
    Aids = np.where(isA)[0]
    Bids = np.where(~isA)[0]
    cntA, cntB = len(Aids), len(Bids)

    pat = masks.astype(np.int64).copy()
    need = 4 - m
    for bbit in range(8):
        unset = (pat >> bbit) & 1 == 0
        add = unset & (need > 0) & isA
        pat += add.astype(np.int64) << bbit
        need = need - add
    pid_vox = PAT_ID[pat]  # valid for A voxels

    # deal A voxels round-robin within each pattern run
    pidA = pid_vox[Aids]
    orderA = np.argsort(pidA, kind="stable")
    sorted_ids = Aids[orderA]
    sorted_pid = pidA[orderA]
    cnts = np.bincount(sorted_pid, minlength=70)
    L = -(-cnts // N_CORES)                      # per-core run length
    offs = np.concatenate([[0], np.cumsum(L)[:-1]])
    starts = np.concatenate([[0], np.cumsum(cnts)[:-1]])
    r = np.arange(cntA) - starts[sorted_pid]
    vox_core = np.empty(num_out, np.int64)
    vox_col = np.empty(num_out, np.int64)
    vox_core[sorted_ids] = r % N_CORES
    vox_col[sorted_ids] = offs[sorted_pid] + r // N_CORES
    NAc = int(L.sum())

    rB = np.arange(cntB)
    vox_core[Bids] = rB % N_CORES
    vox_col[Bids] = NAc + rB // N_CORES
    LB = int(-(-cntB // N_CORES))

    Ntot = NAc + LB
    nb = max(1, -(-Ntot // (16 * PSUM_W)))

    # runs -> per-psum-tile segments
    nT = nb * 16
    segs = [[] for _ in range(nT)]
    for pid in range(70):
        if cnts[pid] == 0:
            continue
        off, run_len = int(offs[pid]), int(L[pid])
        lo = off
        while lo < off + run_len:
            T = lo // PSUM_W
            hi = min(off + run_len, (T + 1) * PSUM_W)
            segs[T].append((0, lo - PSUM_W * T, hi - lo, pid, lo // CHUNK))
            lo = hi
    lo = NAc
    while lo < Ntot:
        T = lo // PSUM_W
        hi = min(Ntot, (T + 1) * PSUM_W)
        segs[T].append((1, lo - PSUM_W * T, hi - lo, lo - NAc))
        lo = hi

    # fill slabs
    pairs = out_idx.astype(np.int64) * 8 + k_idx
    unique_pairs = np.unique(pairs).size == n

    ptA = isA[out_idx]
    xa = x[ptA]
    va = out_idx[ptA]
    ka = k_idx[ptA].astype(np.int64)
    rank = POP[pat[va] & ((np.int64(1) << ka) - 1)]
    slabA = np.zeros((N_CORES, max(NAc, 1), 4, 32), np.float32)
    if unique_pairs:
        slabA[vox_core[va], vox_col[va], rank] = xa
    else:
        np.add.at(slabA, (vox_core[va], vox_col[va], rank), xa)

    xb = x[~ptA]
    vb = out_idx[~ptA]
    kb = k_idx[~ptA].astype(np.int64)
    slabB = np.zeros((N_CORES, max(LB, 1), 8, 32), np.float32)
    if unique_pairs:
        slabB[vox_core[vb], vox_col[vb] - NAc, kb] = xb
    else:
        np.add.at(slabB, (vox_core[vb], vox_col[vb] - NAc, kb), xb)

    # weight dictionary: wdict[32r+c, 32p+d] = W[k_r(p), c, d]
    wd = np.zeros((4, 32, 70, 32), np.float32)
    for p, byte in enumerate(PAT4):
        ks = [k for k in range(8) if (byte >> k) & 1]
        for rr, k in enumerate(ks):
            wd[rr, :, p, :] = W[k]
    wdict = np.ascontiguousarray(wd.reshape(128, 70 * 32)).astype(bf)
    wflat = W.reshape(256, 32).astype(bf)

    in_maps = []
    for c in range(N_CORES):
        in_maps.append({
            "slabA": np.ascontiguousarray(
                slabA[c].reshape(max(NAc, 1), 128).T).astype(bf),
            "slabB": np.ascontiguousarray(
                slabB[c].reshape(max(LB, 1), 256).T).astype(bf),
            "wdict": wdict,
            "wflat": wflat,
        })
    desc = {"NAc": NAc, "LB": LB, "Ntot": Ntot, "nb": nb, "segs": segs}
    return in_maps, desc, vox_core, vox_col


def _decode(results, desc, vox_core, vox_col):
    nb = desc["nb"]
    NT = nb * 16 * PSUM_W
    outs = []
    for rres in results:
        st = np.asarray(rres["out_st"], dtype=np.float32)  # [nb, 128, 2048]
        arr = st.reshape(nb, 4, 32, 4, 512)                # [b, a, d, g, t]
        outT = arr.transpose(2, 0, 3, 1, 4).reshape(32, NT)
        outs.append(outT)
    full = np.stack(outs)                                  # [cores, 32, NT]
    return np.ascontiguousarray(full[vox_core, :, vox_col])


def run(x, W, k_idx, out_idx, num_out, trace=False, dt_name=None):
    from concourse.bass_utils import run_bass_kernel_spmd

    x = np.asarray(x, dtype=np.float32)
    W = np.asarray(W, dtype=np.float32)
    k_idx = np.asarray(k_idx, dtype=np.int32)
    out_idx = np.asarray(out_idx, dtype=np.int32)
    num_out = int(num_out)

    in_maps, desc, vox_core, vox_col = _pack(x, W, k_idx, out_idx, num_out)
    nc = _get_program(desc)
    res = run_bass_kernel_spmd(nc, in_maps, list(range(N_CORES)), trace=trace)
    out = _decode(res.results, desc, vox_core, vox_col)
    return out, res


def kernel(x, W, k_idx, out_idx, num_out):
    out, _ = run(x, W, k_idx, out_idx, num_out, trace=False)
    return out

